# revision 1
# baseline (speedup 1.0000x reference)
"""GCN layer (SpMM) Bass kernel for 8 trn2 NeuronCores.

out[i] = sum_{e: rows[e]==i} edge_vals[e] * embeds[cols[e]]
N=100000 nodes, E=1000000 edges, D=64 features.

Strategy: host sorts edges by destination row and splits nodes into 8
contiguous ranges (12500 nodes/core) with disjoint outputs -> no
collectives. Per core, output rows are processed in blocks of 128; each
block's edges are padded to chunks of 128. Per chunk the device does:
  1. indirect DMA gather   emb[p,:]   = embeds[cols[p], :]      (gpsimd)
  2. scale                 embs[p,:]  = emb[p,:] * vals[p]      (scalar)
  3. one-hot               oh[p,r]    = (rrow[p] == r)          (vector)
  4. matmul accumulate     psum[r,:] += oh.T @ embs             (tensor)
After a block's chunks, PSUM is copied to SBUF and DMA'd to the output
rows (contiguous -> plain DMA, no scatter).

The chunk schedule (chunks per block) is computed from the data on the
host and baked into the program; all 8 cores share one program, so the
per-block chunk count is the max over cores (~4% padding).
"""

import sys

import numpy as np

if "/opt/trn_rl_repo" not in sys.path:
    sys.path.insert(0, "/opt/trn_rl_repo")

N_NODES = 100000
D = 64
P = 128
N_CORES = 8


def _build_program(chunks_per_block, n_chunks, n_nodes, repeats=1):
    import concourse.bacc as bacc
    import concourse.bass as bass
    import concourse.tile as tile
    from concourse import mybir

    nodes_per_core = n_nodes // N_CORES
    n_blocks = len(chunks_per_block)

    nc = bacc.Bacc(
        "TRN2",
        target_bir_lowering=False,
        debug=False,
        num_devices=N_CORES,
    )
    embeds_t = nc.dram_tensor("embeds", [n_nodes, D], mybir.dt.float32, kind="ExternalInput")
    cols_t = nc.dram_tensor("cols_p", [P, n_chunks], mybir.dt.int32, kind="ExternalInput")
    vals_t = nc.dram_tensor("vals_p", [P, n_chunks], mybir.dt.float32, kind="ExternalInput")
    rrow_t = nc.dram_tensor("rrow_p", [P, n_chunks], mybir.dt.float32, kind="ExternalInput")
    iota_t = nc.dram_tensor("iota", [P, P], mybir.dt.float32, kind="ExternalInput")
    out_t = nc.dram_tensor("out", [n_blocks * P, D], mybir.dt.float32, kind="ExternalOutput")

    with tile.TileContext(nc) as tc:
        with (
            tc.tile_pool(name="static", bufs=1) as static_pool,
            tc.tile_pool(name="emb", bufs=8) as emb_pool,
            tc.tile_pool(name="sc", bufs=4) as sc_pool,
            tc.tile_pool(name="oh", bufs=4) as oh_pool,
            tc.tile_pool(name="outp", bufs=4) as out_pool,
            tc.tile_pool(name="psum", bufs=4, space="PSUM") as psum_pool,
        ):
            cols_sb = static_pool.tile([P, n_chunks], mybir.dt.int32)
            vals_sb = static_pool.tile([P, n_chunks], mybir.dt.float32)
            rrow_sb = static_pool.tile([P, n_chunks], mybir.dt.float32)
            iota_sb = static_pool.tile([P, P], mybir.dt.float32)
            nc.sync.dma_start(out=cols_sb[:], in_=cols_t[:])
            nc.sync.dma_start(out=vals_sb[:], in_=vals_t[:])
            nc.sync.dma_start(out=rrow_sb[:], in_=rrow_t[:])
            nc.sync.dma_start(out=iota_sb[:], in_=iota_t[:])

            for _rep in range(repeats):
              j = 0
              for b in range(n_blocks):
                nb = int(chunks_per_block[b])
                psum_tile = psum_pool.tile([P, D], dtype=mybir.dt.float32, space="PSUM")
                for t in range(nb):
                    emb_tile = emb_pool.tile([P, D], mybir.dt.float32)
                    nc.gpsimd.indirect_dma_start(
                        out=emb_tile[:],
                        out_offset=None,
                        in_=embeds_t[:],
                        in_offset=bass.IndirectOffsetOnAxis(
                            ap=cols_sb[:, j : j + 1], axis=0
                        ),
                    )
                    embs_tile = sc_pool.tile([P, D], mybir.dt.float32)
                    nc.scalar.activation(
                        out=embs_tile[:],
                        in_=emb_tile[:],
                        func=mybir.ActivationFunctionType.Copy,
                        scale=vals_sb[:, j : j + 1],
                    )
                    oh_tile = oh_pool.tile([P, P], mybir.dt.float32)
                    nc.vector.tensor_tensor(
                        out=oh_tile[:],
                        in0=rrow_sb[:, j : j + 1].to_broadcast([P, P]),
                        in1=iota_sb[:],
                        op=mybir.AluOpType.is_equal,
                    )
                    nc.tensor.matmul(
                        out=psum_tile[:],
                        lhsT=oh_tile[:],
                        rhs=embs_tile[:],
                        start=(t == 0),
                        stop=(t == nb - 1),
                    )
                    j += 1
                o_sb = out_pool.tile([P, D], mybir.dt.float32)
                nc.scalar.copy(out=o_sb[:], in_=psum_tile[:])
                nc.sync.dma_start(out=out_t[b * P : (b + 1) * P, :], in_=o_sb[:])
    nc.compile()
    return nc


def _kernel_impl(rows, cols, edge_vals, embeds, n_nodes, trace=False):
    from concourse.bass_utils import run_bass_kernel_spmd

    rows = np.asarray(rows).astype(np.int64)
    cs_all = np.asarray(cols).astype(np.int32)
    vs_all = np.asarray(edge_vals).astype(np.float32)
    embeds = np.ascontiguousarray(np.asarray(embeds), dtype=np.float32)

    nodes_per_core = n_nodes // N_CORES
    assert nodes_per_core * N_CORES == n_nodes
    n_blocks = (nodes_per_core + P - 1) // P

    order = np.argsort(rows, kind="stable")
    rs = rows[order]
    cs = cs_all[order]
    vs = vs_all[order]

    core_of_edge = rs // nodes_per_core
    blk_of_edge = (rs - core_of_edge * nodes_per_core) // P
    cnt = np.bincount(
        core_of_edge * n_blocks + blk_of_edge, minlength=N_CORES * n_blocks
    ).reshape(N_CORES, n_blocks)

    chunks_per_block = np.maximum(1, -(-cnt.max(axis=0) // P))  # ceil div
    n_chunks = int(chunks_per_block.sum())
    chunk_base = np.concatenate([[0], np.cumsum(chunks_per_block)])

    cols_p = np.zeros((N_CORES, n_chunks * P), np.int32)
    vals_p = np.zeros((N_CORES, n_chunks * P), np.float32)
    rrow_p = np.zeros((N_CORES, n_chunks * P), np.float32)
    core_edge_bounds = np.searchsorted(rs, np.arange(0, n_nodes + 1, nodes_per_core))
    for k in range(N_CORES):
        e0 = int(core_edge_bounds[k])
        for b in range(n_blocks):
            c = int(cnt[k, b])
            s = int(chunk_base[b]) * P
            cols_p[k, s : s + c] = cs[e0 : e0 + c]
            vals_p[k, s : s + c] = vs[e0 : e0 + c]
            rrow_p[k, s : s + c] = (
                rs[e0 : e0 + c] - k * nodes_per_core - b * P
            ).astype(np.float32)
            e0 += c

    # device layout: [P, n_chunks], partition p / chunk j <- edge j*P+p
    def dev(a, dt):
        return np.ascontiguousarray(
            a.reshape(N_CORES, n_chunks, P).transpose(0, 2, 1)
        ).astype(dt)

    cols_d = dev(cols_p, np.int32)
    vals_d = dev(vals_p, np.float32)
    rrow_d = dev(rrow_p, np.float32)
    iota = np.ascontiguousarray(
        np.tile(np.arange(P, dtype=np.float32), (P, 1))
    )

    nc = _build_program(chunks_per_block, n_chunks, n_nodes)
    in_maps = [
        {
            "embeds": embeds,
            "cols_p": cols_d[k],
            "vals_p": vals_d[k],
            "rrow_p": rrow_d[k],
            "iota": iota,
        }
        for k in range(N_CORES)
    ]
    global _LAST
    _LAST = (nc, in_maps)
    r = run_bass_kernel_spmd(nc, in_maps, list(range(N_CORES)), trace=trace)
    out = np.concatenate(
        [r.results[k]["out"][:nodes_per_core] for k in range(N_CORES)], axis=0
    ).astype(np.float32)
    if trace:
        return out, r
    return out


_LAST = None


def kernel(rows, cols, edge_vals, embeds):
    return _kernel_impl(rows, cols, edge_vals, embeds, N_NODES)



# revision 2
# speedup vs baseline: 1.0223x; 1.0223x over previous
"""GCN layer (SpMM) Bass kernel v2 for 8 trn2 NeuronCores.

out[i] = sum_{e: rows[e]==i} edge_vals[e] * embeds[cols[e]]
N=100000 nodes, E=1000000 edges, D=64 features.

v2 strategy (vs baseline's per-chunk indirect_dma_start):
- rows partitioned across 8 cores (disjoint outputs, no collectives)
- per core, destination blocks of 128 rows; windows of 8 blocks (one PSUM
  bank per block)
- cols split into 4 groups of <=25000 so indices fit dma_gather's int16
- edges ordered (window, group, block, chunk); per (window, group) one
  batched dma_gather (256B/row) instead of hundreds of tiny indirect DMAs
- one-hot scatter matrices built on DVE in bf16 (batched is_equal), embs
  scaled+cast to bf16 on DVE, segment-sum via PE matmul into per-block
  PSUM accumulated across the 4 group passes
- per window: PSUM -> SBUF copies and one DMA to the output rows
"""

import sys

import numpy as np

if "/opt/trn_rl_repo" not in sys.path:
    sys.path.insert(0, "/opt/trn_rl_repo")

from ml_dtypes import bfloat16

N_NODES = 100000
D = 64
P = 128
N_CORES = 8
WBLK = 8          # blocks per window == live PSUM banks
CB = 8            # chunks per DVE batch
SEGMAX = 56       # max chunks per dma_gather call (56*128 = 7168 idxs < 8192)


def _schedule(n_nodes, rows, cols):
    """Shared (all-core) chunk schedule + per-edge slot assignment.

    Returns dict with the schedule and per-core packed arrays.
    """
    npc = n_nodes // N_CORES
    n_blocks = -(-npc // P)
    n_windows = -(-n_blocks // WBLK)
    GS = -(-n_nodes // 4)
    assert GS <= 32767

    k_of = rows // npc
    lr_of = rows - k_of * npc
    b_of = lr_of // P
    g_of = cols // GS

    cnt = np.bincount(
        (k_of * n_blocks + b_of) * 4 + g_of, minlength=N_CORES * n_blocks * 4
    ).reshape(N_CORES, n_blocks, 4)
    cnt_max = cnt.max(axis=0)  # [n_blocks, 4]
    chunks = -(-cnt_max // P)  # ceil
    chunks[:, 0] = np.maximum(chunks[:, 0], 1)  # every block initialized

    # stream order: (w, g, b)
    cell_order = []  # (b, g) in stream order
    for w in range(n_windows):
        bs = range(w * WBLK, min((w + 1) * WBLK, n_blocks))
        for g in range(4):
            for b in bs:
                cell_order.append((b, g))
    cell_pos = np.empty((n_blocks, 4), np.int64)
    cell_chunks = np.empty(len(cell_order), np.int64)
    for i, (b, g) in enumerate(cell_order):
        cell_pos[b, g] = i
        cell_chunks[i] = chunks[b, g]
    chunk_start = np.concatenate([[0], np.cumsum(cell_chunks)])  # per cell pos
    n_chunks = int(chunk_start[-1])
    S = n_chunks * P

    # per-chunk metadata: block slot + first/last flags
    chunk_block = np.empty(n_chunks, np.int64)
    for i, (b, g) in enumerate(cell_order):
        chunk_block[chunk_start[i] : chunk_start[i + 1]] = b
    first_chunk = np.full(n_blocks, -1, np.int64)
    last_chunk = np.zeros(n_blocks, np.int64)
    for c in range(n_chunks):
        b = chunk_block[c]
        if first_chunk[b] < 0:
            first_chunk[b] = c
        last_chunk[b] = c

    # gather segments: per (w, g) runs split to <= SEGMAX chunks
    segments = []  # (g, chunk0, n_chunks_in_call)
    window_chunk0 = []  # first chunk of each window
    ci = 0
    for w in range(n_windows):
        nb_w = min(WBLK, n_blocks - w * WBLK)
        window_chunk0.append(int(chunk_start[ci]))
        for g in range(4):
            seg = int(sum(cell_chunks[ci : ci + nb_w]))
            c0 = int(chunk_start[ci])
            ci += nb_w
            while seg > 0:
                take = min(seg, SEGMAX)
                segments.append((g, c0, take))
                c0 += take
                seg -= take
    window_chunk0.append(n_chunks)

    # per-core slot assignment
    idx16 = np.zeros((N_CORES, S), np.int16)
    vals = np.zeros((N_CORES, S), np.float32)
    rrow = np.full((N_CORES, S), -1.0, np.float32)
    for k in range(N_CORES):
        m = np.flatnonzero(k_of == k)
        cp = cell_pos[b_of[m], g_of[m]]
        o = np.argsort(cp, kind="stable")
        m = m[o]
        cp = cp[o]
        # rank within equal-cp runs
        starts = np.r_[0, np.flatnonzero(np.diff(cp)) + 1]
        lens = np.diff(np.r_[starts, len(cp)])
        ranks = np.arange(len(cp)) - np.repeat(starts, lens)
        slots = chunk_start[cp] * P + ranks
        idx16[k, slots] = (cols[m] - g_of[m] * GS).astype(np.int16)
        vals[k, slots] = 1.0  # overwritten by caller with real vals
        rrow[k, slots] = (lr_of[m] - b_of[m] * P).astype(np.float32)
    return dict(
        npc=npc, n_blocks=n_blocks, n_windows=n_windows, GS=GS,
        n_chunks=n_chunks, S=S, segments=segments,
        chunk_block=chunk_block, first_chunk=first_chunk, last_chunk=last_chunk,
        idx16=idx16, vals=vals, rrow=rrow, k_of=k_of,
        cell_pos=cell_pos, chunk_start=chunk_start, b_of=b_of, g_of=g_of,
        window_chunk0=window_chunk0,
    )


def _build_program(n_nodes, sched, repeats=1):
    import concourse.bacc as bacc
    from concourse import mybir
    import concourse.tile as tile

    n_blocks = sched["n_blocks"]
    n_windows = sched["n_windows"]
    GS = sched["GS"]
    n_chunks = sched["n_chunks"]
    S = sched["S"]
    segments = sched["segments"]
    chunk_block = sched["chunk_block"]
    first_chunk = sched["first_chunk"]
    last_chunk = sched["last_chunk"]

    nc = bacc.Bacc(
        "TRN2",
        target_bir_lowering=False,
        debug=False,
        num_devices=N_CORES,
        num_swdge_queues=4,
    )
    f32, bf16, i16 = mybir.dt.float32, mybir.dt.bfloat16, mybir.dt.int16
    embeds_t = nc.dram_tensor("embeds", [n_nodes, D], f32, kind="ExternalInput")
    idx_t = nc.dram_tensor("idx16", [P, S // 16], i16, kind="ExternalInput")
    vals_t = nc.dram_tensor("vals_p", [P, n_chunks], f32, kind="ExternalInput")
    rrow_t = nc.dram_tensor("rrow_p", [P, n_chunks], bf16, kind="ExternalInput")
    iota_t = nc.dram_tensor("iota", [P, CB * P], bf16, kind="ExternalInput")
    out_t = nc.dram_tensor("out", [n_blocks * P, D], f32, kind="ExternalOutput")

    with tile.TileContext(nc) as tc:
        with (
            tc.tile_pool(name="static", bufs=1) as sp,
            tc.tile_pool(name="gp", bufs=5) as gp,
            tc.tile_pool(name="ohp", bufs=3) as ohp,
            tc.tile_pool(name="ebp", bufs=3) as ebp,
            tc.tile_pool(name="outp", bufs=2) as outp,
            tc.tile_pool(name="psp", bufs=WBLK, space="PSUM") as psp,
        ):
            idx_sb = sp.tile([P, S // 16], i16)
            vals_sb = sp.tile([P, n_chunks], f32)
            rrow_sb = sp.tile([P, n_chunks], bf16)
            iota_sb = sp.tile([P, CB * P], bf16)
            nc.sync.dma_start(out=iota_sb[:], in_=iota_t[:])
            # split metadata loads at window boundaries so the first
            # gathers/matmuls don't wait for the full-stream load
            wc0 = sched["window_chunk0"]
            for w in range(n_windows):
                a, b = wc0[w], wc0[w + 1]
                nc.sync.dma_start(
                    out=idx_sb[:, a * 8 : b * 8], in_=idx_t[:, a * 8 : b * 8]
                )
                nc.sync.dma_start(out=rrow_sb[:, a:b], in_=rrow_t[:, a:b])
                nc.sync.dma_start(out=vals_sb[:, a:b], in_=vals_t[:, a:b])

            for _rep in range(repeats):
                psum_tiles = [None] * WBLK
                out_sb = None
                si = 0
                qn = 0
                for w in range(n_windows):
                    nb_w = min(WBLK, n_blocks - w * WBLK)
                    out_sb = outp.tile([P, nb_w * D], f32, name="osb")
                    # all segments of this window
                    while si < len(segments):
                        g, c0, cs = segments[si]
                        if chunk_block[c0] // WBLK != w:
                            break
                        si += 1
                        gt = gp.tile([P, cs, D], f32, name="gt")
                        nc.gpsimd.dma_gather(
                            out_ap=gt[:],
                            in_ap=embeds_t[g * GS : min((g + 1) * GS, n_nodes), :],
                            idxs_ap=idx_sb[:, c0 * 8 : (c0 + cs) * 8],
                            num_idxs=cs * P,
                            num_idxs_reg=cs * P,
                            elem_size=D,
                            single_packet=False,
                            queue_num=qn % 4,
                        )
                        qn += 1
                        for cbo in range(0, cs, CB):
                            cb = min(CB, cs - cbo)
                            c0b = c0 + cbo
                            oh = ohp.tile([P, cb * P], bf16, name="oh")
                            nc.vector.tensor_tensor(
                                out=oh[:].rearrange("p (c r) -> p c r", c=cb),
                                in0=rrow_sb[:, c0b : c0b + cb].to_broadcast(
                                    [P, cb, P]
                                ),
                                in1=iota_sb[:, : cb * P].rearrange(
                                    "p (c r) -> p c r", c=cb
                                ),
                                op=mybir.AluOpType.is_equal,
                            )
                            eb = ebp.tile([P, cb * D], bf16, name="eb")
                            nc.vector.tensor_tensor(
                                out=eb[:].rearrange("p (c f) -> p c f", c=cb),
                                in0=gt[:, cbo : cbo + cb, :],
                                in1=vals_sb[:, c0b : c0b + cb].to_broadcast(
                                    [P, cb, D]
                                ),
                                op=mybir.AluOpType.mult,
                            )
                            for c in range(cb):
                                cg = c0b + c
                                b = int(chunk_block[cg])
                                slot = b % WBLK
                                start = int(first_chunk[b]) == cg
                                stop = int(last_chunk[b]) == cg
                                if start:
                                    psum_tiles[slot] = psp.tile(
                                        [P, D], dtype=f32, space="PSUM", name="ps"
                                    )
                                nc.tensor.matmul(
                                    out=psum_tiles[slot][:],
                                    lhsT=oh[:, c * P : (c + 1) * P],
                                    rhs=eb[:, c * D : (c + 1) * D],
                                    start=start,
                                    stop=stop,
                                )
                                if stop:
                                    nc.scalar.copy(
                                        out=out_sb[:, slot * D : (slot + 1) * D],
                                        in_=psum_tiles[slot][:],
                                    )
                    # window output: SBUF [p, b, f] -> HBM rows w*WBLK*P + b*P + p
                    dst = out_t[w * WBLK * P : w * WBLK * P + nb_w * P, :]
                    dst3 = dst.rearrange("(b p) f -> p b f", b=nb_w)
                    nc.sync.dma_start(
                        out=dst3,
                        in_=out_sb[:].rearrange("p (b f) -> p b f", b=nb_w),
                    )
    nc.compile()
    return nc


def _kernel_impl(rows, cols, edge_vals, embeds, n_nodes, trace=False, repeats=1):
    from concourse.bass_utils import run_bass_kernel_spmd

    rows = np.asarray(rows).astype(np.int64)
    cols = np.asarray(cols).astype(np.int64)
    vs_all = np.asarray(edge_vals).astype(np.float32)
    embeds = np.ascontiguousarray(np.asarray(embeds), dtype=np.float32)

    sched = _schedule(n_nodes, rows, cols)
    npc = sched["npc"]
    S = sched["S"]
    n_chunks = sched["n_chunks"]

    # fill real edge values into the slot layout (recompute slots like _schedule)
    k_of, b_of, g_of = sched["k_of"], sched["b_of"], sched["g_of"]
    cell_pos, chunk_start = sched["cell_pos"], sched["chunk_start"]
    vals = sched["vals"]
    vals[:] = 0.0
    for k in range(N_CORES):
        m = np.flatnonzero(k_of == k)
        cp = cell_pos[b_of[m], g_of[m]]
        o = np.argsort(cp, kind="stable")
        m = m[o]
        cp = cp[o]
        starts = np.r_[0, np.flatnonzero(np.diff(cp)) + 1]
        lens = np.diff(np.r_[starts, len(cp)])
        ranks = np.arange(len(cp)) - np.repeat(starts, lens)
        slots = chunk_start[cp] * P + ranks
        vals[k, slots] = vs_all[m]

    # device layouts
    idx_dev = np.zeros((N_CORES, P, S // 16), np.int16)
    s = np.arange(S)
    for j in range(8):
        idx_dev[:, 16 * j + (s % 16), s // 16] = sched["idx16"]
    vals_dev = np.zeros((N_CORES, P, n_chunks), np.float32)
    vals_dev[:, s % P, s // P] = vals
    rrow_dev = np.zeros((N_CORES, P, n_chunks), np.float32)
    rrow_dev[:, s % P, s // P] = sched["rrow"]
    rrow_dev = rrow_dev.astype(bfloat16)
    iota = np.tile(np.arange(P, dtype=np.float32), (P, CB)).astype(bfloat16)

    nc = _build_program(n_nodes, sched, repeats=repeats)
    in_maps = [
        {
            "embeds": embeds,
            "idx16": idx_dev[k],
            "vals_p": vals_dev[k],
            "rrow_p": rrow_dev[k],
            "iota": iota,
        }
        for k in range(N_CORES)
    ]
    global _LAST
    _LAST = (nc, in_maps)
    r = run_bass_kernel_spmd(nc, in_maps, list(range(N_CORES)), trace=trace)
    out = np.concatenate(
        [r.results[k]["out"][:npc] for k in range(N_CORES)], axis=0
    ).astype(np.float32)
    if trace:
        return out, r
    return out


_LAST = None


def kernel(rows, cols, edge_vals, embeds):
    return _kernel_impl(rows, cols, edge_vals, embeds, N_NODES)


# revision 3
# speedup vs baseline: 1.0371x; 1.0144x over previous
"""GCN layer (SpMM) Bass kernel v2 for 8 trn2 NeuronCores.

out[i] = sum_{e: rows[e]==i} edge_vals[e] * embeds[cols[e]]
N=100000 nodes, E=1000000 edges, D=64 features.

v2 strategy (vs baseline's per-chunk indirect_dma_start):
- rows partitioned across 8 cores (disjoint outputs, no collectives)
- per core, destination blocks of 128 rows; windows of 8 blocks (one PSUM
  bank per block)
- cols split into 4 groups of <=25000 so indices fit dma_gather's int16
- edges ordered (window, group, block, chunk); per (window, group) one
  batched dma_gather (256B/row) instead of hundreds of tiny indirect DMAs
- one-hot scatter matrices built on DVE in bf16 (batched is_equal), embs
  scaled+cast to bf16 on DVE, segment-sum via PE matmul into per-block
  PSUM accumulated across the 4 group passes
- per window: PSUM -> SBUF copies and one DMA to the output rows
"""

import sys

import numpy as np

if "/opt/trn_rl_repo" not in sys.path:
    sys.path.insert(0, "/opt/trn_rl_repo")

from ml_dtypes import bfloat16

N_NODES = 100000
D = 64
P = 128
N_CORES = 8
WBLK = 8          # blocks per window == live PSUM banks
CB = 8            # chunks per DVE batch
SEGMAX = 56       # max chunks per dma_gather call (56*128 = 7168 idxs < 8192)


def _schedule(n_nodes, rows, cols):
    """Shared (all-core) chunk schedule + per-edge slot assignment.

    Returns dict with the schedule and per-core packed arrays.
    """
    npc = n_nodes // N_CORES
    n_blocks = -(-npc // P)
    n_windows = -(-n_blocks // WBLK)
    GS = -(-n_nodes // 4)
    assert GS <= 32767

    k_of = rows // npc
    lr_of = rows - k_of * npc
    b_of = lr_of // P
    g_of = cols // GS

    cnt = np.bincount(
        (k_of * n_blocks + b_of) * 4 + g_of, minlength=N_CORES * n_blocks * 4
    ).reshape(N_CORES, n_blocks, 4)
    cnt_max = cnt.max(axis=0)  # [n_blocks, 4]
    chunks = -(-cnt_max // P)  # ceil
    chunks[:, 0] = np.maximum(chunks[:, 0], 1)  # every block initialized

    # stream order: (w, g, b)
    cell_order = []  # (b, g) in stream order
    for w in range(n_windows):
        bs = range(w * WBLK, min((w + 1) * WBLK, n_blocks))
        for g in range(4):
            for b in bs:
                cell_order.append((b, g))
    cell_pos = np.empty((n_blocks, 4), np.int64)
    cell_chunks = np.empty(len(cell_order), np.int64)
    for i, (b, g) in enumerate(cell_order):
        cell_pos[b, g] = i
        cell_chunks[i] = chunks[b, g]
    chunk_start = np.concatenate([[0], np.cumsum(cell_chunks)])  # per cell pos
    n_chunks = int(chunk_start[-1])
    S = n_chunks * P

    # per-chunk metadata: block slot + first/last flags
    chunk_block = np.empty(n_chunks, np.int64)
    for i, (b, g) in enumerate(cell_order):
        chunk_block[chunk_start[i] : chunk_start[i + 1]] = b
    first_chunk = np.full(n_blocks, -1, np.int64)
    last_chunk = np.zeros(n_blocks, np.int64)
    for c in range(n_chunks):
        b = chunk_block[c]
        if first_chunk[b] < 0:
            first_chunk[b] = c
        last_chunk[b] = c

    # gather segments: per (w, g) runs split to <= SEGMAX chunks
    segments = []  # (g, chunk0, n_chunks_in_call)
    window_chunk0 = []  # first chunk of each window
    ci = 0
    for w in range(n_windows):
        nb_w = min(WBLK, n_blocks - w * WBLK)
        window_chunk0.append(int(chunk_start[ci]))
        for g in range(4):
            seg = int(sum(cell_chunks[ci : ci + nb_w]))
            c0 = int(chunk_start[ci])
            ci += nb_w
            while seg > 0:
                take = min(seg, SEGMAX)
                segments.append((g, c0, take))
                c0 += take
                seg -= take
    window_chunk0.append(n_chunks)

    # per-core slot assignment
    idx16 = np.zeros((N_CORES, S), np.int16)
    vals = np.zeros((N_CORES, S), np.float32)
    rrow = np.full((N_CORES, S), -1.0, np.float32)
    for k in range(N_CORES):
        m = np.flatnonzero(k_of == k)
        cp = cell_pos[b_of[m], g_of[m]]
        o = np.argsort(cp, kind="stable")
        m = m[o]
        cp = cp[o]
        # rank within equal-cp runs
        starts = np.r_[0, np.flatnonzero(np.diff(cp)) + 1]
        lens = np.diff(np.r_[starts, len(cp)])
        ranks = np.arange(len(cp)) - np.repeat(starts, lens)
        slots = chunk_start[cp] * P + ranks
        idx16[k, slots] = (cols[m] - g_of[m] * GS).astype(np.int16)
        vals[k, slots] = 1.0  # overwritten by caller with real vals
        rrow[k, slots] = (lr_of[m] - b_of[m] * P).astype(np.float32)
    return dict(
        npc=npc, n_blocks=n_blocks, n_windows=n_windows, GS=GS,
        n_chunks=n_chunks, S=S, segments=segments,
        chunk_block=chunk_block, first_chunk=first_chunk, last_chunk=last_chunk,
        idx16=idx16, vals=vals, rrow=rrow, k_of=k_of,
        cell_pos=cell_pos, chunk_start=chunk_start, b_of=b_of, g_of=g_of,
        window_chunk0=window_chunk0,
    )


def _build_program(n_nodes, sched, repeats=1):
    import concourse.bacc as bacc
    from concourse import mybir
    import concourse.tile as tile

    n_blocks = sched["n_blocks"]
    n_windows = sched["n_windows"]
    GS = sched["GS"]
    n_chunks = sched["n_chunks"]
    S = sched["S"]
    segments = sched["segments"]
    chunk_block = sched["chunk_block"]
    first_chunk = sched["first_chunk"]
    last_chunk = sched["last_chunk"]

    nc = bacc.Bacc(
        "TRN2",
        target_bir_lowering=False,
        debug=False,
        num_devices=N_CORES,
        num_swdge_queues=4,
    )
    f32, bf16, i16 = mybir.dt.float32, mybir.dt.bfloat16, mybir.dt.int16
    embeds_t = nc.dram_tensor("embeds", [n_nodes, D], f32, kind="ExternalInput")
    idx_t = nc.dram_tensor("idx16", [P, S // 16], i16, kind="ExternalInput")
    vals_t = nc.dram_tensor("vals_p", [P, n_chunks], f32, kind="ExternalInput")
    rrow_t = nc.dram_tensor("rrow_p", [P, n_chunks], bf16, kind="ExternalInput")
    iota_t = nc.dram_tensor("iota", [P, CB * P], bf16, kind="ExternalInput")
    out_t = nc.dram_tensor("out", [n_blocks * P, D], f32, kind="ExternalOutput")

    with tile.TileContext(nc) as tc:
        with (
            tc.tile_pool(name="static", bufs=1) as sp,
            tc.tile_pool(name="gp", bufs=5) as gp,
            tc.tile_pool(name="ohp", bufs=3) as ohp,
            tc.tile_pool(name="ebp", bufs=3) as ebp,
            tc.tile_pool(name="outp", bufs=2) as outp,
            tc.tile_pool(name="psp", bufs=WBLK, space="PSUM") as psp,
        ):
            idx_sb = sp.tile([P, S // 16], i16)
            vals_sb = sp.tile([P, n_chunks], f32)
            rrow_sb = sp.tile([P, n_chunks], bf16)
            iota_sb = sp.tile([P, CB * P], bf16)
            nc.sync.dma_start(out=iota_sb[:], in_=iota_t[:])
            # split metadata loads at window boundaries so the first
            # gathers/matmuls don't wait for the full-stream load
            wc0 = sched["window_chunk0"]
            for w in range(n_windows):
                a, b = wc0[w], wc0[w + 1]
                nc.sync.dma_start(
                    out=idx_sb[:, a * 8 : b * 8], in_=idx_t[:, a * 8 : b * 8]
                )
                nc.sync.dma_start(out=rrow_sb[:, a:b], in_=rrow_t[:, a:b])
                nc.sync.dma_start(out=vals_sb[:, a:b], in_=vals_t[:, a:b])

            for _rep in range(repeats):
                psum_tiles = [None] * WBLK
                out_sb = None
                si = 0
                qn = 0
                for w in range(n_windows):
                    nb_w = min(WBLK, n_blocks - w * WBLK)
                    out_sb = outp.tile([P, nb_w * D], f32, name="osb")
                    # all segments of this window
                    while si < len(segments):
                        g, c0, cs = segments[si]
                        if chunk_block[c0] // WBLK != w:
                            break
                        si += 1
                        gt = gp.tile([P, cs, D], f32, name="gt")
                        nc.gpsimd.dma_gather(
                            out_ap=gt[:],
                            in_ap=embeds_t[g * GS : min((g + 1) * GS, n_nodes), :],
                            idxs_ap=idx_sb[:, c0 * 8 : (c0 + cs) * 8],
                            num_idxs=cs * P,
                            num_idxs_reg=cs * P,
                            elem_size=D,
                            single_packet=False,
                            queue_num=qn % 4,
                        )
                        qn += 1
                        for cbo in range(0, cs, CB):
                            cb = min(CB, cs - cbo)
                            c0b = c0 + cbo
                            oh = ohp.tile([P, cb * P], bf16, name="oh")
                            nc.vector.tensor_tensor(
                                out=oh[:].rearrange("p (c r) -> p c r", c=cb),
                                in0=rrow_sb[:, c0b : c0b + cb].to_broadcast(
                                    [P, cb, P]
                                ),
                                in1=iota_sb[:, : cb * P].rearrange(
                                    "p (c r) -> p c r", c=cb
                                ),
                                op=mybir.AluOpType.is_equal,
                            )
                            eb = ebp.tile([P, cb * D], bf16, name="eb")
                            nc.vector.tensor_tensor(
                                out=eb[:].rearrange("p (c f) -> p c f", c=cb),
                                in0=gt[:, cbo : cbo + cb, :],
                                in1=vals_sb[:, c0b : c0b + cb].to_broadcast(
                                    [P, cb, D]
                                ),
                                op=mybir.AluOpType.mult,
                            )
                            for c in range(cb):
                                cg = c0b + c
                                b = int(chunk_block[cg])
                                slot = b % WBLK
                                start = int(first_chunk[b]) == cg
                                stop = int(last_chunk[b]) == cg
                                if start:
                                    psum_tiles[slot] = psp.tile(
                                        [P, D], dtype=f32, space="PSUM", name="ps"
                                    )
                                nc.tensor.matmul(
                                    out=psum_tiles[slot][:],
                                    lhsT=oh[:, c * P : (c + 1) * P],
                                    rhs=eb[:, c * D : (c + 1) * D],
                                    start=start,
                                    stop=stop,
                                )
                                if stop:
                                    nc.scalar.copy(
                                        out=out_sb[:, slot * D : (slot + 1) * D],
                                        in_=psum_tiles[slot][:],
                                    )
                    # window output: SBUF [p, b, f] -> HBM rows w*WBLK*P + b*P + p
                    dst = out_t[w * WBLK * P : w * WBLK * P + nb_w * P, :]
                    dst3 = dst.rearrange("(b p) f -> p b f", b=nb_w)
                    # ACT-ring HWDGE: keeps output writes off the sync ring
                    # that streams the metadata loads
                    nc.scalar.dma_start(
                        out=dst3,
                        in_=out_sb[:].rearrange("p (b f) -> p b f", b=nb_w),
                    )
    nc.compile()
    return nc


def _kernel_impl(rows, cols, edge_vals, embeds, n_nodes, trace=False, repeats=1):
    from concourse.bass_utils import run_bass_kernel_spmd

    rows = np.asarray(rows).astype(np.int64)
    cols = np.asarray(cols).astype(np.int64)
    vs_all = np.asarray(edge_vals).astype(np.float32)
    embeds = np.ascontiguousarray(np.asarray(embeds), dtype=np.float32)

    sched = _schedule(n_nodes, rows, cols)
    npc = sched["npc"]
    S = sched["S"]
    n_chunks = sched["n_chunks"]

    # fill real edge values into the slot layout (recompute slots like _schedule)
    k_of, b_of, g_of = sched["k_of"], sched["b_of"], sched["g_of"]
    cell_pos, chunk_start = sched["cell_pos"], sched["chunk_start"]
    vals = sched["vals"]
    vals[:] = 0.0
    for k in range(N_CORES):
        m = np.flatnonzero(k_of == k)
        cp = cell_pos[b_of[m], g_of[m]]
        o = np.argsort(cp, kind="stable")
        m = m[o]
        cp = cp[o]
        starts = np.r_[0, np.flatnonzero(np.diff(cp)) + 1]
        lens = np.diff(np.r_[starts, len(cp)])
        ranks = np.arange(len(cp)) - np.repeat(starts, lens)
        slots = chunk_start[cp] * P + ranks
        vals[k, slots] = vs_all[m]

    # device layouts
    idx_dev = np.zeros((N_CORES, P, S // 16), np.int16)
    s = np.arange(S)
    for j in range(8):
        idx_dev[:, 16 * j + (s % 16), s // 16] = sched["idx16"]
    vals_dev = np.zeros((N_CORES, P, n_chunks), np.float32)
    vals_dev[:, s % P, s // P] = vals
    rrow_dev = np.zeros((N_CORES, P, n_chunks), np.float32)
    rrow_dev[:, s % P, s // P] = sched["rrow"]
    rrow_dev = rrow_dev.astype(bfloat16)
    iota = np.tile(np.arange(P, dtype=np.float32), (P, CB)).astype(bfloat16)

    nc = _build_program(n_nodes, sched, repeats=repeats)
    in_maps = [
        {
            "embeds": embeds,
            "idx16": idx_dev[k],
            "vals_p": vals_dev[k],
            "rrow_p": rrow_dev[k],
            "iota": iota,
        }
        for k in range(N_CORES)
    ]
    global _LAST
    _LAST = (nc, in_maps)
    r = run_bass_kernel_spmd(nc, in_maps, list(range(N_CORES)), trace=trace)
    out = np.concatenate(
        [r.results[k]["out"][:npc] for k in range(N_CORES)], axis=0
    ).astype(np.float32)
    if trace:
        return out, r
    return out


_LAST = None


def kernel(rows, cols, edge_vals, embeds):
    return _kernel_impl(rows, cols, edge_vals, embeds, N_NODES)
